# revision 4
# baseline (speedup 1.0000x reference)
"""Trainium2 Bass kernel for nn_DiseaseLevelShapleyGame.

Math: shapley = 0.5*(v_sim + t_sim)*labels where
  v_sim = normalize(x @ Wv.T + bv) @ normalize(prototypes).T
Key identity: v_sim = proj / max(||v_pre||, eps) with
  proj[k,b] = pn[k] . v_pre[b],  ||v_pre||^2 = sum_j v_pre[j,b]^2,
so the device only needs, per batch column b: the 14 prototype projections and
the squared row norm of v_pre — the full [B, 4096] v_pre never leaves SBUF.

Device scheme (per core, batch shard of 2048, data-parallel over 8 cores):
out[j, b] = Wv.T-chunk.T @ x.T-chunk accumulated over k (f32r matmuls, N=512).
Per j-tile of 128 output features: add bias (per-partition scalar on DVE),
square on ACT, then two accumulating matmuls fold the j-dimension away:
  proj  += pnT[j-tile].T @ v_pre_tile      ([14, 512] PSUM, accumulates over j)
  norm2 += ones.T       @ v_pre_tile^2     ([1, 512] PSUM)
Final divide/sqrt/labels/Gram-matrix/loss are tiny and run on the host.
"""
import sys

if "/opt/trn_rl_repo" not in sys.path:
    sys.path.insert(0, "/opt/trn_rl_repo")

import numpy as np

import concourse.bass as bass
import concourse.mybir as mybir
import concourse.tile as tile
from concourse import bacc
from concourse.bass import ds
from concourse.bass_utils import run_bass_kernel_spmd

B, D, K = 16384, 4096, 14
NC = 8
BC = B // NC            # 2048 batch rows per core
KO = D // 128           # 32 contraction chunks
NJ = D // 128           # 32 output-feature tiles
NSUP = 2                # batch super-blocks per core
SUPW = BC // NSUP       # 1024
BB = 512                # matmul moving free dim
NBB = SUPW // BB        # 2
EPS = 1e-12
F32 = mybir.dt.float32
F32R = mybir.dt.float32r

_NC_CACHE = None


def _build_nc():
    nc = bacc.Bacc(None, target_bir_lowering=False)
    x_d = nc.dram_tensor("x", [KO, 128, BC], F32R, kind="ExternalInput")
    t_d = nc.dram_tensor("t", [KO, 128, BC], F32R, kind="ExternalInput")
    wv_d = nc.dram_tensor("wv", [NJ, 128, KO, 128], F32R, kind="ExternalInput")
    wt_d = nc.dram_tensor("wt", [NJ, 128, KO, 128], F32R, kind="ExternalInput")
    pn_d = nc.dram_tensor("pn", [128, NJ, K], F32R, kind="ExternalInput")
    bv_d = nc.dram_tensor("bv", [128, NJ], F32, kind="ExternalInput")
    bt_d = nc.dram_tensor("bt", [128, NJ], F32, kind="ExternalInput")
    ov_d = nc.dram_tensor("out_v", [K + 1, BC], F32, kind="ExternalOutput")
    ot_d = nc.dram_tensor("out_t", [K + 1, BC], F32, kind="ExternalOutput")

    with tile.TileContext(nc) as tc:
        with tc.tile_pool(name="const", bufs=1) as cpool, \
             tc.tile_pool(name="xp", bufs=1) as xpool, \
             tc.tile_pool(name="wp", bufs=2) as wpool, \
             tc.tile_pool(name="vp", bufs=3) as vpool, \
             tc.tile_pool(name="op", bufs=2) as opool, \
             tc.tile_pool(name="psv", bufs=2, space="PSUM") as pp, \
             tc.tile_pool(name="psacc", bufs=1, space="PSUM") as pacc:
            pn_sb = cpool.tile([128, NJ, K], F32R)
            nc.sync.dma_start(pn_sb[:], pn_d[:])
            bv_sb = cpool.tile([128, NJ], F32)
            nc.sync.dma_start(bv_sb[:], bv_d[:])
            bt_sb = cpool.tile([128, NJ], F32)
            nc.sync.dma_start(bt_sb[:], bt_d[:])
            ones_f32 = cpool.tile([128, 1], F32)
            nc.any.memset(ones_f32[:], 1.0)
            ones_sb = cpool.tile([128, 1], F32R)
            nc.vector.tensor_copy(ones_sb[:], ones_f32[:])

            for xd, wd, bias_sb, outd in ((x_d, wv_d, bv_sb, ov_d),
                                          (t_d, wt_d, bt_sb, ot_d)):
                for s in range(NSUP):
                    xs = []
                    for bb in range(NBB):
                        xt = xpool.tile([128, KO, BB], F32R, tag=f"xs{bb}")
                        nc.sync.dma_start(
                            xt[:],
                            xd[:, :, ds(s * SUPW + bb * BB, BB)].rearrange(
                                "ko ki b -> ki ko b"),
                        )
                        xs.append(xt)
                    psn = [pacc.tile([1, BB], F32, tag=f"psn{bb}",
                                     name=f"psn{bb}") for bb in range(NBB)]
                    psp = [pacc.tile([K, BB], F32, tag=f"psp{bb}",
                                     name=f"psp{bb}") for bb in range(NBB)]
                    for jt in range(NJ):
                        w_sb = wpool.tile([128, KO, 128], F32R, tag="w")
                        nc.sync.dma_start(w_sb[:], wd[jt])
                        for bb in range(NBB):
                            psv = pp.tile([128, BB], F32, tag="psv")
                            for ko in range(KO):
                                nc.tensor.matmul(psv[:], w_sb[:, ko, :],
                                                 xs[bb][:, ko, :],
                                                 start=(ko == 0),
                                                 stop=(ko == KO - 1))
                            v_sb = vpool.tile([128, BB], F32R, tag="vsb")
                            nc.vector.tensor_scalar_add(v_sb[:], psv[:],
                                                        bias_sb[:, jt:jt + 1])
                            vsq = vpool.tile([128, BB], F32R, tag="vsq")
                            nc.scalar.square(vsq[:], v_sb[:])
                            nc.tensor.matmul(psp[bb][:], pn_sb[:, jt, :], v_sb[:],
                                             start=(jt == 0), stop=(jt == NJ - 1))
                            nc.tensor.matmul(psn[bb][:], ones_sb[:], vsq[:],
                                             start=(jt == 0), stop=(jt == NJ - 1))
                    for bb in range(NBB):
                        stp = opool.tile([K, BB], F32, tag="stp")
                        stn = opool.tile([1, BB], F32, tag="stn")
                        nc.any.tensor_copy(stp[:], psp[bb][:])
                        nc.any.tensor_copy(stn[:], psn[bb][:])
                        nc.sync.dma_start(
                            outd[:K, ds(s * SUPW + bb * BB, BB)], stp[:])
                        nc.sync.dma_start(
                            outd[K:K + 1, ds(s * SUPW + bb * BB, BB)], stn[:])
    nc.compile()
    return nc


def _get_nc():
    global _NC_CACHE
    if _NC_CACHE is None:
        _NC_CACHE = _build_nc()
    return _NC_CACHE


def _prep_in_maps(image_feat, text_feat, Wv, bv, Wt, bt, prototypes):
    pn = prototypes / np.maximum(
        np.sqrt(np.sum(prototypes.astype(np.float64) ** 2, axis=1, keepdims=True)),
        EPS).astype(np.float64)
    pn = pn.astype(np.float32)
    pn_swz = np.ascontiguousarray(
        pn.T.reshape(NJ, 128, K).transpose(1, 0, 2))          # [128, NJ, K]

    def w_swz(W):
        # [jt, ki, ko, j_in]: lhsT tile (k on partitions, j on free)
        return np.ascontiguousarray(
            W.T.reshape(KO, 128, NJ, 128).transpose(2, 1, 0, 3))

    wv_s = w_swz(Wv)
    wt_s = w_swz(Wt)
    bv_s = np.ascontiguousarray(bv.reshape(NJ, 128).T)        # [128, NJ]
    bt_s = np.ascontiguousarray(bt.reshape(NJ, 128).T)

    in_maps = []
    for c in range(NC):
        sl = slice(c * BC, (c + 1) * BC)
        xc = np.ascontiguousarray(image_feat[sl].T).reshape(KO, 128, BC)
        tc_ = np.ascontiguousarray(text_feat[sl].T).reshape(KO, 128, BC)
        in_maps.append({
            "x": xc, "t": tc_, "wv": wv_s, "wt": wt_s,
            "pn": pn_swz, "bv": bv_s, "bt": bt_s,
        })
    return in_maps


def _postprocess(results, disease_labels, prototypes, cooccurrence):
    out_v = np.concatenate([r["out_v"] for r in results], axis=1)  # [15, B]
    out_t = np.concatenate([r["out_t"] for r in results], axis=1)
    labels_f = disease_labels.astype(np.float32)

    def sim(out):
        proj = out[:K].astype(np.float64)                  # [14, B]
        norm = np.sqrt(np.maximum(out[K].astype(np.float64), 0.0))
        return (proj / np.maximum(norm, EPS)[None, :]).T   # [B, 14]

    shapley = (0.5 * (sim(out_v) + sim(out_t))).astype(np.float32) * labels_f
    batch_cooccur = (labels_f.T @ labels_f) / np.float32(B)
    learned = 1.0 / (1.0 + np.exp(-cooccurrence.astype(np.float64)))
    cooccur_loss = np.float32(np.mean((learned - batch_cooccur.astype(np.float64)) ** 2))
    return shapley, prototypes.astype(np.float32), cooccur_loss


def kernel(image_feat, text_feat, disease_labels, Wv, bv, Wt, bt,
           prototypes, cooccurrence):
    image_feat = np.asarray(image_feat, dtype=np.float32)
    text_feat = np.asarray(text_feat, dtype=np.float32)
    disease_labels = np.asarray(disease_labels)
    Wv = np.asarray(Wv, dtype=np.float32)
    bv = np.asarray(bv, dtype=np.float32)
    Wt = np.asarray(Wt, dtype=np.float32)
    bt = np.asarray(bt, dtype=np.float32)
    prototypes = np.asarray(prototypes, dtype=np.float32)
    cooccurrence = np.asarray(cooccurrence, dtype=np.float32)

    nc = _get_nc()
    in_maps = _prep_in_maps(image_feat, text_feat, Wv, bv, Wt, bt, prototypes)
    results = run_bass_kernel_spmd(nc, in_maps, list(range(NC))).results
    return _postprocess(results, disease_labels, prototypes, cooccurrence)
